# revision 10
# baseline (speedup 1.0000x reference)
"""Trainium2 Bass kernel for nn_Attention_3d (B=1, C=64, D=48, W=128, H=128).

The reference factorizes into 64*48 independent 128x128 attentions (one per
(channel, depth)): S_c = Q_c K_c^T over h, softmax over rows, O_c = A_c V_c,
plus 1x1 convs over channels before/after.

Sharding: depth D split across 8 cores (6 slices each), weights replicated.

Per-core pipeline (per depth slice d), all matmuls 16-bit:
  A. QK conv: stationary = x w-pair chunks [(wpar,c), h] fp16, moving =
     blockdiag([wq'|wk]^T) fp16 split hi+lo (two accumulating matmuls,
     fp32-exact weights) -> PSUM [h, (wpar, qk, c)].
     Evac: Q + bias (DVE tensor_add, bias tile) -> qbuf fp16; K plain copy
     (Pool) -> kbuf fp16.  K's conv bias is dropped entirely: it only
     contributes a per-column factor to exp(S^T) which cancels in softmax.
  B. V conv: stationary = x h-pair chunks [(hpar,c), w] fp16, moving =
     blockdiag(wv^T) fp16 -> PSUM [w, (hpar, c)] -> vbuf fp16 (ACT copy).
     V's bias is folded into the final conv bias (host side).
  C. Scores: per channel: S^T = kbuf_c^T @ qbuf_c -> PSUM [v, w] f32;
     exp(S^T - 45) on ACT (batched 4 channels) -> ebuf bf16.
  D. AV + denominators: per channel: lhsT = ebuf_c, rhs = vbuf_c (fp16)
     -> PSUM [w, h] (unnormalized); plus N=1 matmul vs ones -> den[w, c].
     Evac unnormalized -> obuf f32 (Pool copy).
  E. R tile: rr = 1/den (DVE reciprocal), duplicated, PE-transposed via
     moving [I|I] -> R[(hpar,c), w] f32 -> R4 sbuf.
  F. O^T: per h-pair chunk: regular matmul obuf-chunk(f32r) vs [I|I]
     -> PSUM [(hpar,c), w]; evac = DVE tensor_mul with R4 (normalization
     fused) -> och fp16.
  G. Final conv: wobd fp16 stationary, och moving (N=512) -> PSUM
     -> ybuf fp16 (Pool copy) -> DMA.  Output bias (incl. folded V bias)
     added on host during decode.
"""
import time
import numpy as np
import ml_dtypes
from contextlib import ExitStack

B, C, D, W, H = 1, 64, 48, 128, 128
NCORES = 8
DLOC = D // NCORES  # 6
QSCALE = float(8.0 ** -0.25)
ESHIFT = -45.0

_f32 = np.float32
_f16 = np.float16
_bf16 = ml_dtypes.bfloat16


def round_fp32r(x):
    u = np.ascontiguousarray(x, dtype=np.float32).view(np.uint32)
    lsb = (u >> 12) & 1
    u = (u + 0x7FF + lsb) & np.uint32(0xFFFFF000)
    return u.view(np.float32)


def _build_bass():
    import concourse.bacc as bacc
    import concourse.mybir as mybir
    import concourse.tile as tile

    fr = mybir.dt.float32r
    f32 = mybir.dt.float32
    bf = mybir.dt.bfloat16
    fh = mybir.dt.float16
    AF = mybir.ActivationFunctionType

    nc = bacc.Bacc("TRN2", target_bir_lowering=False, debug=False)

    xw_d = nc.dram_tensor("xw", [128, DLOC * 64 * 128], fh, kind="ExternalInput").ap()
    xh_d = nc.dram_tensor("xh", [128, DLOC * 64 * 128], fh, kind="ExternalInput").ap()
    wqk_d = nc.dram_tensor("wqk", [128, 512], fh, kind="ExternalInput").ap()
    wvbd_d = nc.dram_tensor("wvbd", [128, 128], fh, kind="ExternalInput").ap()
    wobd_d = nc.dram_tensor("wobd", [128, 128], fh, kind="ExternalInput").ap()
    ii_d = nc.dram_tensor("ii", [128, 256], fr, kind="ExternalInput").ap()
    qbias_d = nc.dram_tensor("qbias", [128, 512], f32, kind="ExternalInput").ap()
    y_d = nc.dram_tensor("y", [DLOC, 128, 64, 128], fh, kind="ExternalOutput").ap()

    with tile.TileContext(nc) as tc, ExitStack() as ctx:
        const = ctx.enter_context(tc.tile_pool(name="const", bufs=1))
        xwpool = ctx.enter_context(tc.tile_pool(name="xwpool", bufs=2))
        xhpool = ctx.enter_context(tc.tile_pool(name="xhpool", bufs=2))
        sbpool = ctx.enter_context(tc.tile_pool(name="sbpool", bufs=1))

        ppool = ctx.enter_context(tc.tile_pool(name="ppool", bufs=7, space="PSUM"))
        pden = ctx.enter_context(tc.tile_pool(name="pden", bufs=1, space="PSUM"))

        def ptile():
            return ppool.tile([128, 512], f32, name="pp")

        # constants
        wqk_sb = const.tile([128, 512], fh)
        nc.gpsimd.dma_start(wqk_sb[:], wqk_d[:])
        wvbd_sb = const.tile([128, 128], fh)
        nc.gpsimd.dma_start(wvbd_sb[:], wvbd_d[:])
        wobd_sb = const.tile([128, 128], fh)
        nc.gpsimd.dma_start(wobd_sb[:], wobd_d[:])
        ii_sb = const.tile([128, 256], fr)
        nc.gpsimd.dma_start(ii_sb[:], ii_d[:])
        qbias_sb = const.tile([128, 2, 2, 128], f32)
        nc.gpsimd.dma_start(
            qbias_sb[:], qbias_d[:].rearrange("p (a b c) -> p a b c", a=2, b=2)
        )
        ones_bf = const.tile([128, 1], bf)
        nc.gpsimd.memset(ones_bf[:], 1.0)
        eshift_sb = const.tile([128, 1], f32)
        nc.gpsimd.memset(eshift_sb[:], ESHIFT)

        for d in range(DLOC):
            xw_sb = xwpool.tile([128, 8192], fh)
            nc.gpsimd.dma_start(xw_sb[:, 0:4096], xw_d[:, d * 8192:d * 8192 + 4096])
            nc.gpsimd.dma_start(xw_sb[:, 4096:8192],
                                xw_d[:, d * 8192 + 4096:(d + 1) * 8192])
            xh_sb = xhpool.tile([128, 8192], fh)
            nc.gpsimd.dma_start(xh_sb[:, 0:4096], xh_d[:, d * 8192:d * 8192 + 4096])
            nc.gpsimd.dma_start(xh_sb[:, 4096:8192],
                                xh_d[:, d * 8192 + 4096:(d + 1) * 8192])

            qkbuf = sbpool.tile([128, 128, 128], fh, tag="qkbuf")  # [h, (qk,c), w]
            vbuf = sbpool.tile([128, 64, 128], fh, tag="vbuf")   # [v, c, h]
            ebuf = sbpool.tile([128, 64, 128], bf, tag="ebuf")   # [v, c, w]
            obuf = sbpool.tile([128, 64, 2, 64], fr, tag="obuf")  # [w, hp, hpar, c]
            och = sbpool.tile([128, 64, 128], fh, tag="och")     # [(hpar,c), hp, w]
            ybuf = sbpool.tile([128, 64, 128], fh, tag="ybuf")   # [(hpar,o), hp, w]
            rr2 = sbpool.tile([128, 2, 64], fr, tag="rr2")      # [w, 2, c]
            r4 = sbpool.tile([128, 2, 128], f32, tag="r4")       # [(hpar,c), 2, w]

            # ---- A. QK conv (2 w-pairs per PSUM bank; Whi+Wlo accumulate)
            for wp in range(64):
                if wp % 2 == 0:
                    pc = ptile().rearrange("p (a b) -> p a b", a=2)
                sl = pc[:, wp % 2, :]
                lhs = xw_sb[:, wp * 128:(wp + 1) * 128]
                nc.tensor.matmul(sl, lhs, wqk_sb[:, 0:256], start=True, stop=False)
                nc.tensor.matmul(sl, lhs, wqk_sb[:, 256:512], start=False, stop=True)
                if wp % 2 == 1:
                    # psum [h, (wp2, par2, qkc128)]
                    pr = pc.rearrange("p a (b qc) -> p a b qc", b=2)
                    w0 = 2 * (wp - 1)
                    dqk = qkbuf[:, :, w0:w0 + 4].rearrange(
                        "p q (a b) -> p a b q", a=2)
                    nc.vector.tensor_add(dqk, pr, qbias_sb[:])

            # ---- B. V conv (4 h-pairs per bank)
            for hp in range(64):
                if hp % 4 == 0:
                    pv = ptile().rearrange("p (a b) -> p a b", a=4)
                nc.tensor.matmul(
                    pv[:, hp % 4, :], xh_sb[:, hp * 128:(hp + 1) * 128],
                    wvbd_sb[:], start=True, stop=True,
                )
                if hp % 4 == 3:
                    h0 = 2 * (hp - 3)
                    dv = vbuf[:, :, h0:h0 + 8].rearrange(
                        "p c (a b) -> p a b c", a=4)
                    nc.scalar.copy(dv, pv.rearrange("p a (b c) -> p a b c", b=2))

            # ---- C/D. scores + exp + AV + den, software-pipelined
            den = pden.tile([128, 64], f32)

            def issue_scores(g):
                ps = ptile().rearrange("p (a b) -> p a b", a=4)
                for j in range(4):
                    c = 4 * g + j
                    nc.tensor.matmul(ps[:, j, :], qkbuf[:, 64 + c, :],
                                     qkbuf[:, c, :], start=True, stop=True)
                nc.scalar.activation(ebuf[:, 4 * g:4 * g + 4, :], ps,
                                     AF.Exp, bias=eshift_sb[:])

            def issue_av(g):
                pa = ptile().rearrange("p (a b) -> p a b", a=4)
                for j in range(4):
                    c = 4 * g + j
                    nc.tensor.matmul(pa[:, j, :], ebuf[:, c, :], vbuf[:, c, :],
                                     start=True, stop=True)
                    nc.tensor.matmul(den[:, c:c + 1], ebuf[:, c, :],
                                     ones_bf[:, 0:1], start=True, stop=True)
                do = obuf[:, :, :, 4 * g:4 * g + 4].rearrange(
                    "p hp par c -> p c (hp par)")
                with nc.allow_low_precision(reason="fp32r obuf for PE transpose"):
                    nc.vector.tensor_copy(do, pa)

            issue_scores(0)
            for g in range(1, 16):
                issue_scores(g)
                issue_av(g - 1)
            issue_av(15)

            # ---- E. reciprocal + R tile
            with nc.allow_low_precision(reason="fp32r rr for PE transpose"):
                nc.vector.reciprocal(rr2[:, 0, :], den[:])
                nc.vector.tensor_copy(rr2[:, 1, :], rr2[:, 0, :])
            prr = ptile()
            nc.tensor.matmul(
                prr[:, 0:256], rr2[:].rearrange("p a c -> p (a c)"),
                ii_sb[:], start=True, stop=True,
            )
            nc.vector.tensor_copy(r4[:, 0, :], prr[:, 0:128])
            nc.vector.tensor_copy(r4[:, 1, :], r4[:, 0, :])

            # ---- F/G. O^T transposes + final conv, interleaved
            for t in range(0, 64, 2):
                pt = ptile().rearrange("p (a b) -> p a b", a=2)
                for j in (0, 1):
                    nc.tensor.matmul(
                        pt[:, j, :],
                        obuf[:, t + j, :, :].rearrange("p a c -> p (a c)"),
                        ii_sb[:], start=True, stop=True,
                    )
                nc.vector.tensor_mul(och[:, t:t + 2, :], pt[:, :, 0:128], r4[:])
                if t % 8 == 6:
                    gg = t // 8
                    py = ptile()
                    nc.tensor.matmul(py[:], wobd_sb[:],
                                     och[:, 8 * gg:8 * gg + 4, :].rearrange(
                                         "p a b -> p (a b)"),
                                     start=True, stop=True)
                    nc.scalar.copy(
                        ybuf[:, 8 * gg:8 * gg + 4, :].rearrange("p a b -> p (a b)"),
                        py[:])
                    py2 = ptile()
                    nc.tensor.matmul(py2[:], wobd_sb[:],
                                     och[:, 8 * gg + 4:8 * gg + 8, :].rearrange(
                                         "p a b -> p (a b)"),
                                     start=True, stop=True)
                    nc.scalar.copy(
                        ybuf[:, 8 * gg + 4:8 * gg + 8, :].rearrange("p a b -> p (a b)"),
                        py2[:])

            for q in range(4):
                nc.gpsimd.dma_start(
                    y_d[d, :, 16 * q:16 * (q + 1), :],
                    ybuf[:, 16 * q:16 * q + 16, :],
                )

    nc.compile()
    return nc


def _prep_inputs(x, wq, bq, wk, bk, wv, bv, wo, bo):
    """Build per-core input maps (host-side layouts, fp16)."""
    x = np.asarray(x, _f32)[0]           # [64, 48, 128, 128]
    wq2 = np.asarray(wq, _f32) * QSCALE
    bq2 = np.asarray(bq, _f32) * QSCALE
    wk = np.asarray(wk, _f32)
    wv = np.asarray(wv, _f32)
    bv = np.asarray(bv, _f32)
    wo = np.asarray(wo, _f32)
    bo = np.asarray(bo, _f32)

    A = np.concatenate([wq2.T, wk.T], axis=1)       # [64c, 128=(q,k)]
    wqk32 = np.zeros((128, 256), _f32)
    wqk32[0:64, 0:128] = A
    wqk32[64:128, 128:256] = A
    wqk_hi = wqk32.astype(_f16)
    wqk_lo = (wqk32 - wqk_hi.astype(_f32)).astype(_f16)
    wqk = np.concatenate([wqk_hi, wqk_lo], axis=1)  # [128, 512]

    wvbd = np.zeros((128, 128), _f16)
    wvbd[0:64, 0:64] = wv.T.astype(_f16)
    wvbd[64:128, 64:128] = wv.T.astype(_f16)
    wobd = np.zeros((128, 128), _f16)
    wobd[0:64, 0:64] = wo.T.astype(_f16)
    wobd[64:128, 64:128] = wo.T.astype(_f16)

    ii = np.concatenate([np.eye(128, dtype=_f32)] * 2, axis=1)  # [128, 256]
    ii = round_fp32r(ii)

    bqk_row = np.concatenate([bq2, np.zeros(64, _f32)])     # Q bias | K zero
    qbias = np.ascontiguousarray(
        np.broadcast_to(bqk_row[None, None, None, :], (128, 2, 2, 128))
        .reshape(128, 512), dtype=_f32,
    )

    in_maps = []
    for i in range(NCORES):
        xc = x[:, i * DLOC:(i + 1) * DLOC]          # [64, 6, 128, 128]
        xw = np.empty((128, DLOC, 64, 128), _f16)
        xw[0:64] = xc[:, :, 0::2, :]
        xw[64:128] = xc[:, :, 1::2, :]
        xt = xc.transpose(0, 1, 3, 2)               # [c, d, h, w]
        xh = np.empty((128, DLOC, 64, 128), _f16)
        xh[0:64] = xt[:, :, 0::2, :]
        xh[64:128] = xt[:, :, 1::2, :]
        in_maps.append({
            "xw": np.ascontiguousarray(xw.reshape(128, DLOC * 64 * 128)),
            "xh": np.ascontiguousarray(xh.reshape(128, DLOC * 64 * 128)),
            "wqk": wqk, "wvbd": wvbd, "wobd": wobd,
            "ii": ii, "qbias": qbias,
        })
    return in_maps


_BO2 = None


def _decode_outputs(results, bo2):
    """results: per-core dicts with 'y' [6, 128, 64, 128] fp16.

    y[d, (hpar, o), hp, w] -> [o, d, w, h], h = 2*hp + hpar; add bias.
    """
    outs = []
    for r in results:
        y = np.asarray(r["y"]).reshape(DLOC, 2, 64, 64, 128)
        Y = y.transpose(2, 0, 4, 3, 1).reshape(64, DLOC, 128, 128)
        outs.append(Y.astype(_f32))
    full = np.concatenate(outs, axis=1)             # [64, 48, 128, 128]
    full += bo2[:, None, None, None]
    return full[None]


_CACHE = {}


def _get_runner():
    """Build bass program + cached jitted pjrt callable."""
    if "runner" in _CACHE:
        return _CACHE["runner"]
    import jax
    from jax.sharding import Mesh, PartitionSpec
    from jax.experimental.shard_map import shard_map
    import concourse.mybir as mybir
    from concourse import bass2jax
    from concourse.bass2jax import _bass_exec_p, install_neuronx_cc_hook

    install_neuronx_cc_hook()
    nc = _build_bass()

    partition_name = (
        nc.partition_id_tensor.name if nc.partition_id_tensor else None
    )
    in_names, out_names, out_avals = [], [], []
    for alloc in nc.m.functions[0].allocations:
        if not isinstance(alloc, mybir.MemoryLocationSet):
            continue
        name = alloc.memorylocations[0].name
        if alloc.kind == "ExternalInput":
            if name != partition_name:
                in_names.append(name)
        elif alloc.kind == "ExternalOutput":
            out_names.append(name)
            out_avals.append(
                jax.core.ShapedArray(
                    tuple(alloc.tensor_shape), mybir.dt.np(alloc.dtype)
                )
            )
    n_params = len(in_names)
    zero_shapes = [(a.shape, a.dtype) for a in out_avals]
    all_in_names = list(in_names) + list(out_names)
    if partition_name is not None:
        all_in_names.append(partition_name)

    def _body(*args):
        operands = list(args)
        if partition_name is not None:
            operands.append(bass2jax.partition_id_tensor())
        outs = _bass_exec_p.bind(
            *operands,
            out_avals=tuple(out_avals),
            in_names=tuple(all_in_names),
            out_names=tuple(out_names),
            lowering_input_output_aliases=(),
            sim_require_finite=True,
            sim_require_nnan=True,
            nc=nc,
        )
        return tuple(outs)

    devices = jax.devices()[:NCORES]
    mesh = Mesh(np.asarray(devices), ("core",))
    n_outs = len(out_names)
    in_specs = (PartitionSpec("core"),) * (n_params + n_outs)
    out_specs = (PartitionSpec("core"),) * n_outs
    donate = tuple(range(n_params, n_params + n_outs))
    sharded = jax.jit(
        shard_map(_body, mesh=mesh, in_specs=in_specs, out_specs=out_specs,
                  check_rep=False),
        donate_argnums=donate,
        keep_unused=True,
    )

    def run(in_maps):
        concat_in = [
            np.concatenate([np.asarray(in_maps[c][nm]) for c in range(NCORES)],
                           axis=0)
            for nm in in_names
        ]
        concat_zeros = [
            np.zeros((NCORES * s[0],) + tuple(s[1:]), dt)
            for (s, dt) in zero_shapes
        ]
        out = sharded(*concat_in, *concat_zeros)
        res = []
        for c in range(NCORES):
            res.append({
                nm: np.asarray(out[i]).reshape(NCORES, *zero_shapes[i][0])[c]
                for i, nm in enumerate(out_names)
            })
        return res, (sharded, in_names, zero_shapes, out_names)

    _CACHE["runner"] = run
    return run


def kernel(**inputs):
    run = _get_runner()
    in_maps = _prep_inputs(**inputs)
    bo2 = (np.asarray(inputs["bo"], _f32)
           + np.asarray(inputs["wo"], _f32) @ np.asarray(inputs["bv"], _f32))
    results, _ = run(in_maps)
    return _decode_outputs(results, bo2)


if __name__ == "__main__":
    import reference
    t0 = time.time()
    ins = {k: np.asarray(v) for k, v in reference.setup_inputs().items()}
    exp = np.asarray(reference.reference(**ins))
    t1 = time.time()
    print(f"reference: {t1 - t0:.1f}s", flush=True)
    act = kernel(**ins)
    t2 = time.time()
    print(f"kernel: {t2 - t1:.1f}s", flush=True)
    err = np.abs(act - exp)
    scale = np.abs(exp).mean()
    print(f"abs err max={err.max():.3e} mean={err.mean():.3e} "
          f"rel(max/scale)={err.max() / scale:.3e} "
          f"rel_mean={(err / (np.abs(exp) + 1e-6)).mean():.3e}", flush=True)
